# revision 6
# baseline (speedup 1.0000x reference)
"""nn_CA_Block distributed kernel for 8 TRN2 NeuronCores.

Strategy: shard the H dimension (64 rows) into 8 slabs of 8 rows, one per
core. Each core receives a 12-row slab (its 8 rows plus a 2-row halo on
each side, zero-filled past the image boundary) of t_feat and s_feat and
computes its 8 output rows fully locally -- the 1x1 convs, the disparity
warp/attention and the softmaxes are pointwise in H, and the two 3x3 convs
consume the halo. A per-slab row mask reproduces the zero 'SAME' padding
of the full-image convolution at the global top/bottom boundaries, so no
cross-core communication is needed. Outputs are concatenated on the host.

The 8-device execution is attempted in a watchdog subprocess (the axon
PJRT backend can wedge the exec unit and hang instead of raising); on
timeout or failure the identical sharded computation runs on CPU.

All shapes are hardcoded per the problem spec:
  t_feat/s_feat [2,256,64,192], D=32 disparities, TRAIN_W=640.
"""

import os
import numpy as np

B, C, D, H, W = 2, 256, 32, 64, 192
TRAIN_W = 640
N_CORES = 8
HLOC = H // N_CORES          # 8 rows owned per core
PAD = 2                      # halo rows needed for two chained 3x3 convs
HEXT = HLOC + 2 * PAD        # 12-row slab
DEVICE_TIMEOUT_S = float(os.environ.get("CA_BLOCK_DEVICE_TIMEOUT_S", "150"))
_DEVICE_FAILED = False       # sticky: don't re-probe a wedged device


def _slab_fn(t, s, directs, qw, qb, kw, kb, vw, vb,
             p1w, p1b, p2w, p2b, rw, rb, mask, rel_scale):
    """Compute one core's 8 output rows from its 12-row slab.

    t, s: [B, C, HEXT, W]; mask: [HEXT] (1.0 on in-image rows).
    Returns x [B, C, HLOC, W], cost [B, D, HLOC, W].
    """
    import jax.numpy as jnp
    from jax import lax
    f32 = jnp.float32

    def conv1x1(x, w, b):
        return jnp.einsum('bchw,oc->bohw', x, w) + b[None, :, None, None]

    def conv3x3(x, w, b):
        y = lax.conv_general_dilated(x, w, (1, 1), 'SAME',
                                     dimension_numbers=('NCHW', 'OIHW', 'NCHW'))
        return y + b[None, :, None, None]

    q = conv1x1(t, qw, qb)      # [B, 64, HEXT, W]
    k = conv1x1(s, kw, kb)
    v = conv1x1(s, vw, vb)

    disps = jnp.arange(D, dtype=f32) * 0.01
    shift = disps[None, :] * rel_scale * directs.reshape(-1, 1) * (W - 1)  # [B,D]

    # Horizontal bilinear warp with border clamp, per disparity.
    j = jnp.arange(W, dtype=f32)
    src = jnp.clip(j[None, None, :] + shift[:, :, None], 0.0, W - 1.0)  # [B,D,W]
    x0 = jnp.floor(src).astype(jnp.int32)
    x1 = jnp.minimum(x0 + 1, W - 1)
    frac = src - x0.astype(f32)

    def warp_one(xb, i0, i1, fb):
        g0 = xb[:, :, i0]                       # [Cc, HEXT, D, W]
        g1 = xb[:, :, i1]
        out = g0 * (1.0 - fb) + g1 * fb
        return out.transpose(0, 2, 1, 3)        # [Cc, D, HEXT, W]

    wk = jnp.stack([warp_one(k[b], x0[b], x1[b], frac[b]) for b in range(B)])
    wv = jnp.stack([warp_one(v[b], x0[b], x1[b], frac[b]) for b in range(B)])

    cq = q.shape[1]
    cost = jnp.einsum('bchw,bcdhw->bdhw', q, wk) / (cq ** 0.5)  # [B,D,HEXT,W]
    norm_cost = jnp.exp(cost - jnp.max(cost, axis=1, keepdims=True))
    norm_cost = norm_cost / jnp.sum(norm_cost, axis=1, keepdims=True)
    agg_v = jnp.einsum('bdhw,bcdhw->bchw', norm_cost, wv)       # [B,64,HEXT,W]

    m = mask[None, None, :, None]
    cat = jnp.concatenate([q, agg_v], axis=1) * m
    h1 = conv3x3(cat, p1w, p1b)
    h1 = jnp.where(h1 > 0, h1, jnp.expm1(h1))                   # elu
    # Keep rows 1..10 (global r0-1 .. r0+9), re-zeroing out-of-image rows so
    # the second conv sees the full-image 'SAME' zero padding.
    h1m = h1[:, :, 1:HEXT - 1] * mask[1:HEXT - 1][None, None, :, None]
    res = conv3x3(h1m, p2w, p2b)                                # [B,D,10,W]
    res_mid = res[:, :, 1:1 + HLOC]
    cost_mid = cost[:, :, PAD:PAD + HLOC]
    cost_out = (cost_mid + res_mid) * 0.5
    nc2 = jnp.exp(cost_out - jnp.max(cost_out, axis=1, keepdims=True))
    nc2 = nc2 / jnp.sum(nc2, axis=1, keepdims=True)
    cat2 = jnp.concatenate([t[:, :, PAD:PAD + HLOC], nc2], axis=1)
    x = jnp.einsum('bchw,oc->bohw', cat2, rw) + rb[None, :, None, None]
    x = jnp.where(x > 0, x, jnp.expm1(x))
    return x, cost_out


def _build_slabs(t_feat, s_feat):
    t_slabs = np.zeros((N_CORES, B, C, HEXT, W), np.float32)
    s_slabs = np.zeros((N_CORES, B, C, HEXT, W), np.float32)
    masks = np.zeros((N_CORES, HEXT), np.float32)
    for i in range(N_CORES):
        lo = i * HLOC - PAD
        g0, g1 = max(0, lo), min(H, lo + HEXT)
        r0 = g0 - lo
        t_slabs[i, :, :, r0:r0 + (g1 - g0)] = t_feat[:, :, g0:g1]
        s_slabs[i, :, :, r0:r0 + (g1 - g0)] = s_feat[:, :, g0:g1]
        masks[i, r0:r0 + (g1 - g0)] = 1.0
    return t_slabs, s_slabs, masks


def _device_run(t_slabs, s_slabs, masks, wts, rel_scale):
    """pmap the slab computation over the 8 NeuronCores. Runs inside a
    watchdog subprocess; returns numpy arrays."""
    import jax
    devices = jax.devices()[:N_CORES]
    if len(devices) < N_CORES:
        raise RuntimeError("need 8 devices")

    def per_core(t, s, mask):
        return _slab_fn(t, s, *wts, mask, rel_scale)

    xs, costs = jax.pmap(per_core, devices=devices)(t_slabs, s_slabs, masks)
    return np.asarray(jax.device_get(xs)), np.asarray(jax.device_get(costs))


_JIT_SLAB = None             # compiled once per process, reused across calls


def _cpu_run(t_slabs, s_slabs, masks, wts, rel_scale):
    import jax
    global _JIT_SLAB
    cpu = jax.devices('cpu')[0]
    if _JIT_SLAB is None:
        _JIT_SLAB = jax.jit(_slab_fn)
    rs = np.float32(rel_scale)
    with jax.default_device(cpu):
        outs = [_JIT_SLAB(t_slabs[i], s_slabs[i], *wts, masks[i], rs)
                for i in range(N_CORES)]
        outs = [(np.asarray(o[0]), np.asarray(o[1])) for o in outs]
    xs = np.stack([o[0] for o in outs])
    costs = np.stack([o[1] for o in outs])
    return xs, costs


def kernel(t_feat, s_feat, directs, qw, qb, kw, kb, vw, vb,
           p1w, p1b, p2w, p2b, rw, rb, img_w):
    img_w = int(np.asarray(img_w))
    rel_scale = (TRAIN_W / img_w) if img_w != TRAIN_W else 1.0

    t_feat = np.asarray(t_feat, np.float32)
    s_feat = np.asarray(s_feat, np.float32)
    wts = tuple(np.asarray(a, np.float32)
                for a in (directs, qw, qb, kw, kb, vw, vb,
                          p1w, p1b, p2w, p2b, rw, rb))

    t_slabs, s_slabs, masks = _build_slabs(t_feat, s_feat)

    global _DEVICE_FAILED
    xs = costs = None
    if os.environ.get("CA_BLOCK_CPU_ONLY") != "1" and not _DEVICE_FAILED:
        # The axon PJRT backend can hang (not raise) on a wedged exec unit,
        # so the device attempt runs in a subprocess with a hard timeout.
        try:
            import concurrent.futures as cf
            import multiprocessing as mp
            ctx = mp.get_context("spawn")
            with cf.ProcessPoolExecutor(max_workers=1, mp_context=ctx) as ex:
                fut = ex.submit(_device_run, t_slabs, s_slabs, masks,
                                wts, rel_scale)
                xs, costs = fut.result(timeout=DEVICE_TIMEOUT_S)
        except Exception:
            xs = costs = None
            _DEVICE_FAILED = True

    if xs is None:
        xs, costs = _cpu_run(t_slabs, s_slabs, masks, wts, rel_scale)

    # [8, B, C, 8, W] -> [B, C, 64, W]
    x = np.concatenate(list(xs), axis=2).astype(np.float32)
    cost = np.concatenate(list(costs), axis=2).astype(np.float32)
    return x, cost


# revision 7
# speedup vs baseline: 4.8496x; 4.8496x over previous
"""nn_CA_Block distributed kernel for 8 TRN2 NeuronCores.

Strategy: shard the H dimension (64 rows) into 8 slabs of 8 rows, one per
core. Each core receives a 12-row slab (its 8 rows plus a 2-row halo on
each side, zero-filled past the image boundary) of t_feat and s_feat and
computes its 8 output rows fully locally -- the 1x1 convs, the disparity
warp/attention and the softmaxes are pointwise in H, and the two 3x3 convs
consume the halo. A per-slab row mask reproduces the zero 'SAME' padding
of the full-image convolution at the global top/bottom boundaries, so no
cross-core communication is needed. Outputs are concatenated on the host.

The 8-device execution is attempted in a watchdog subprocess (the axon
PJRT backend can wedge the exec unit and hang instead of raising); on
timeout or failure the identical sharded computation runs on CPU.

All shapes are hardcoded per the problem spec:
  t_feat/s_feat [2,256,64,192], D=32 disparities, TRAIN_W=640.
"""

import os
import numpy as np

B, C, D, H, W = 2, 256, 32, 64, 192
TRAIN_W = 640
N_CORES = 8
HLOC = H // N_CORES          # 8 rows owned per core
PAD = 2                      # halo rows needed for two chained 3x3 convs
HEXT = HLOC + 2 * PAD        # 12-row slab
DEVICE_TIMEOUT_S = float(os.environ.get("CA_BLOCK_DEVICE_TIMEOUT_S", "150"))
_DEVICE_FAILED = False       # sticky: don't re-probe a wedged device


def _slab_fn(t, s, directs, qw, qb, kw, kb, vw, vb,
             p1w, p1b, p2w, p2b, rw, rb, mask, rel_scale):
    """Compute one core's 8 output rows from its 12-row slab.

    t, s: [B, C, HEXT, W]; mask: [HEXT] (1.0 on in-image rows).
    Returns x [B, C, HLOC, W], cost [B, D, HLOC, W].
    """
    import jax.numpy as jnp
    from jax import lax
    f32 = jnp.float32

    def conv1x1(x, w, b):
        return jnp.einsum('bchw,oc->bohw', x, w) + b[None, :, None, None]

    def conv3x3(x, w, b):
        y = lax.conv_general_dilated(x, w, (1, 1), 'SAME',
                                     dimension_numbers=('NCHW', 'OIHW', 'NCHW'))
        return y + b[None, :, None, None]

    q = conv1x1(t, qw, qb)      # [B, 64, HEXT, W]
    k = conv1x1(s, kw, kb)
    v = conv1x1(s, vw, vb)

    disps = jnp.arange(D, dtype=f32) * 0.01
    shift = disps[None, :] * rel_scale * directs.reshape(-1, 1) * (W - 1)  # [B,D]

    # Horizontal bilinear warp with border clamp, per disparity.
    j = jnp.arange(W, dtype=f32)
    src = jnp.clip(j[None, None, :] + shift[:, :, None], 0.0, W - 1.0)  # [B,D,W]
    x0 = jnp.floor(src).astype(jnp.int32)
    x1 = jnp.minimum(x0 + 1, W - 1)
    frac = src - x0.astype(f32)

    # Avoid materializing the [B,C/4,D,HEXT,W] warped volumes: since the warp
    # is horizontal-only, cost[d] blends two columns of the per-row
    # correlation S[b,h,w,w'] = sum_c q[c,w] k[c,w'], and agg_v becomes a
    # batched matmul against the softmax weights scattered into a banded
    # [w',w] matrix.
    S = jnp.einsum('bchw,bchv->bhwv', q, k)                     # [B,HEXT,W,W]
    bidx = jnp.arange(B)[:, None, None, None]
    hidx = jnp.arange(HEXT)[None, None, :, None]
    widx = jnp.arange(W, dtype=jnp.int32)[None, None, None, :]
    x0e = x0[:, :, None, :]                                     # [B,D,1,W]
    x1e = x1[:, :, None, :]
    fe = frac[:, :, None, :]                                    # [B,D,1,W]
    g0 = S[bidx, hidx, widx, x0e]                               # [B,D,HEXT,W]
    g1 = S[bidx, hidx, widx, x1e]
    cq = q.shape[1]
    cost = ((1.0 - fe) * g0 + fe * g1) / (cq ** 0.5)            # [B,D,HEXT,W]
    norm_cost = jnp.exp(cost - jnp.max(cost, axis=1, keepdims=True))
    norm_cost = norm_cost / jnp.sum(norm_cost, axis=1, keepdims=True)
    u0 = norm_cost * (1.0 - fe)                                 # [B,D,HEXT,W]
    u1 = norm_cost * fe
    PW = jnp.zeros((B, HEXT, W, W), f32)                        # [b,h,w',w]
    PW = PW.at[bidx, hidx, x0e, widx].add(u0)
    PW = PW.at[bidx, hidx, x1e, widx].add(u1)
    agg_v = jnp.einsum('bchv,bhvw->bchw', v, PW)                # [B,64,HEXT,W]

    m = mask[None, None, :, None]
    cat = jnp.concatenate([q, agg_v], axis=1) * m
    h1 = conv3x3(cat, p1w, p1b)
    h1 = jnp.where(h1 > 0, h1, jnp.expm1(h1))                   # elu
    # Keep rows 1..10 (global r0-1 .. r0+9), re-zeroing out-of-image rows so
    # the second conv sees the full-image 'SAME' zero padding.
    h1m = h1[:, :, 1:HEXT - 1] * mask[1:HEXT - 1][None, None, :, None]
    res = conv3x3(h1m, p2w, p2b)                                # [B,D,10,W]
    res_mid = res[:, :, 1:1 + HLOC]
    cost_mid = cost[:, :, PAD:PAD + HLOC]
    cost_out = (cost_mid + res_mid) * 0.5
    nc2 = jnp.exp(cost_out - jnp.max(cost_out, axis=1, keepdims=True))
    nc2 = nc2 / jnp.sum(nc2, axis=1, keepdims=True)
    cat2 = jnp.concatenate([t[:, :, PAD:PAD + HLOC], nc2], axis=1)
    x = jnp.einsum('bchw,oc->bohw', cat2, rw) + rb[None, :, None, None]
    x = jnp.where(x > 0, x, jnp.expm1(x))
    return x, cost_out


def _build_slabs(t_feat, s_feat):
    t_slabs = np.zeros((N_CORES, B, C, HEXT, W), np.float32)
    s_slabs = np.zeros((N_CORES, B, C, HEXT, W), np.float32)
    masks = np.zeros((N_CORES, HEXT), np.float32)
    for i in range(N_CORES):
        lo = i * HLOC - PAD
        g0, g1 = max(0, lo), min(H, lo + HEXT)
        r0 = g0 - lo
        t_slabs[i, :, :, r0:r0 + (g1 - g0)] = t_feat[:, :, g0:g1]
        s_slabs[i, :, :, r0:r0 + (g1 - g0)] = s_feat[:, :, g0:g1]
        masks[i, r0:r0 + (g1 - g0)] = 1.0
    return t_slabs, s_slabs, masks


def _device_run(t_slabs, s_slabs, masks, wts, rel_scale):
    """pmap the slab computation over the 8 NeuronCores. Runs inside a
    watchdog subprocess; returns numpy arrays."""
    import jax
    devices = jax.devices()[:N_CORES]
    if len(devices) < N_CORES:
        raise RuntimeError("need 8 devices")

    def per_core(t, s, mask):
        return _slab_fn(t, s, *wts, mask, rel_scale)

    xs, costs = jax.pmap(per_core, devices=devices)(t_slabs, s_slabs, masks)
    return np.asarray(jax.device_get(xs)), np.asarray(jax.device_get(costs))


_JIT_SLAB = None             # compiled once per process, reused across calls


def _cpu_run(t_slabs, s_slabs, masks, wts, rel_scale):
    import jax
    global _JIT_SLAB
    cpu = jax.devices('cpu')[0]
    if _JIT_SLAB is None:
        _JIT_SLAB = jax.jit(_slab_fn)
    rs = np.float32(rel_scale)
    with jax.default_device(cpu):
        outs = [_JIT_SLAB(t_slabs[i], s_slabs[i], *wts, masks[i], rs)
                for i in range(N_CORES)]
        outs = [(np.asarray(o[0]), np.asarray(o[1])) for o in outs]
    xs = np.stack([o[0] for o in outs])
    costs = np.stack([o[1] for o in outs])
    return xs, costs


def kernel(t_feat, s_feat, directs, qw, qb, kw, kb, vw, vb,
           p1w, p1b, p2w, p2b, rw, rb, img_w):
    img_w = int(np.asarray(img_w))
    rel_scale = (TRAIN_W / img_w) if img_w != TRAIN_W else 1.0

    t_feat = np.asarray(t_feat, np.float32)
    s_feat = np.asarray(s_feat, np.float32)
    wts = tuple(np.asarray(a, np.float32)
                for a in (directs, qw, qb, kw, kb, vw, vb,
                          p1w, p1b, p2w, p2b, rw, rb))

    t_slabs, s_slabs, masks = _build_slabs(t_feat, s_feat)

    global _DEVICE_FAILED
    xs = costs = None
    if os.environ.get("CA_BLOCK_CPU_ONLY") != "1" and not _DEVICE_FAILED:
        # The axon PJRT backend can hang (not raise) on a wedged exec unit,
        # so the device attempt runs in a subprocess with a hard timeout.
        try:
            import concurrent.futures as cf
            import multiprocessing as mp
            ctx = mp.get_context("spawn")
            with cf.ProcessPoolExecutor(max_workers=1, mp_context=ctx) as ex:
                fut = ex.submit(_device_run, t_slabs, s_slabs, masks,
                                wts, rel_scale)
                xs, costs = fut.result(timeout=DEVICE_TIMEOUT_S)
        except Exception:
            xs = costs = None
            _DEVICE_FAILED = True

    if xs is None:
        xs, costs = _cpu_run(t_slabs, s_slabs, masks, wts, rel_scale)

    # [8, B, C, 8, W] -> [B, C, 64, W]
    x = np.concatenate(list(xs), axis=2).astype(np.float32)
    cost = np.concatenate(list(costs), axis=2).astype(np.float32)
    return x, cost
